# revision 4
# baseline (speedup 1.0000x reference)
"""DiSAN Trainium2 Bass kernel v4 — 8-core data parallel (one example per core).

S/T (directional-softmax denominator and h-weighted numerator) via
zero-padded mask WEIGHTS + PSUM accumulation: a group of 16 queries shares
one [m=128, 32] weight matrix (query j's fw/bw mask pair in columns 2j:2j+2,
zeros elsewhere, built on-device by an int32 stride-17 diag copy); 16
matmuls — one per query, each streaming that query's [z | z*h] 400 columns —
accumulate into one [32, 400] PSUM region. Zero-weight columns add exact
zeros, so the region ends up densely packed [2q+dir, (S|T)] with no
extraction. Groups stack at partition bases {0, 32, 64} of a bank, giving
[96, 400] tiles for vectorized softmax post (s = T/S with the all-masked
uniform fallback, reciprocal_approx_fast). This replaces v1's 4
LDWEIGHTS+MATMUL pairs per query with 1, ~3.5x less tensor time.

Pipeline: z-production (hrep broadcast MMs -> G add -> tanh -> exp -> z*h)
runs 2 chunks ahead of the S/T matmuls (SKEW) so the tensor queue always has
ready work; z*h alternates Vector/GpSimd. The packed s is PE-transposed into
sT (fp32 + bf16 copies); the fusion gate runs per-ST-tile in bf16
(Wf1/Wf2/Ws1/Ws and all gate transposes bf16) so most of it overlaps the
other block's main loop. Head: feat columns as exact bf16 hi/lo weight pairs
against bf16 F1 into one [2, 200] accumulation, then y = sum(relu(y1)*F2row).

kernel(**inputs) takes the full unsharded inputs and returns the full (8,)
output; batch is sharded across the 8 NeuronCores via run_bass_kernel_spmd.
"""

from contextlib import ExitStack

import numpy as np
import ml_dtypes

import concourse.bass as bass
import concourse.bacc as bacc
import concourse.tile as tile
from concourse import mybir

F32 = mybir.dt.float32
BF16 = mybir.dt.bfloat16
I32 = mybir.dt.int32
AF = mybir.ActivationFunctionType
ALU = mybir.AluOpType
AX = mybir.AxisListType

L = 128          # sequence length
D = 200          # feature dim
DC = 100         # feature chunk (2 chunks of 100)
VOCAB = 32000
PAD = 1
N_CORES = 8
CHUNK_I = 8      # queries per G/z chunk
N_CHUNKS = L // CHUNK_I   # 16
GQ = 16          # queries per S/T accumulation group (2 chunks)
C_VAL = 5.0


def build_nc():
    nc = bacc.Bacc("TRN2", target_bir_lowering=False, debug=False)

    def din(name, shape, dt):
        return nc.dram_tensor(name, shape, dt, kind="ExternalInput").ap()

    x_idx_d = {"c": din("xc_idx", [L, 1], I32), "r": din("xr_idx", [L, 1], I32)}
    emb = din("emb", [VOCAB, D], F32)
    Wh = din("Wh", [D, D], F32)
    W1 = din("W1", [D, D], F32)
    W2 = din("W2", [D, D], F32)
    Wf1 = din("Wf1", [D, D], BF16)
    Wf2 = din("Wf2", [D, D], BF16)
    Ws1 = din("Ws1", [2 * D, 2 * D], BF16)
    Ws = din("Ws", [2 * D, 2 * D], BF16)
    F1bf = din("F1bf", [DC, 16, D], BF16)
    F2row = din("F2row", [1, D], F32)
    b_rep = din("b_rep", [L, D], F32)
    masks_d = {"c": din("masks_c", [L, 2 * L], BF16),
               "r": din("masks_r", [L, 2 * L], BF16)}
    ident_f = din("ident_f", [L, L], F32)
    ident_b = din("ident_b", [L, L], BF16)

    y_out = nc.dram_tensor("y", [1, 1], F32, kind="ExternalOutput").ap()

    # SBUF->SBUF flatten needs no DRAM scratch
    with tile.TileContext(nc) as tc, ExitStack() as ctx:
        singles = ctx.enter_context(tc.tile_pool(name="singles", bufs=1))
        blockp = ctx.enter_context(tc.tile_pool(name="blockp", bufs=2))
        work = ctx.enter_context(tc.tile_pool(name="work", bufs=2))
        sml = ctx.enter_context(tc.tile_pool(name="sml", bufs=2))
        zp = ctx.enter_context(tc.tile_pool(name="zp", bufs=4))
        ps_hrep = ctx.enter_context(tc.tile_pool(name="ps_hrep", bufs=2, space="PSUM"))
        ps_st = ctx.enter_context(tc.tile_pool(name="ps_st", bufs=1, space="PSUM"))
        ps_mm = ctx.enter_context(tc.tile_pool(name="ps_mm", bufs=2, space="PSUM"))

        def _t(pool, shape, dt, tag, **kw):
            return pool.tile(shape, dt, name=tag, tag=tag, **kw)

        _dmaq = [nc.sync, nc.scalar, nc.gpsimd]
        _dmaqi = [0]

        def spread_dma(out, in_):
            eng = _dmaq[_dmaqi[0] % len(_dmaq)]
            _dmaqi[0] += 1
            eng.dma_start(out=out, in_=in_)

        def load(ap_dram, shape, dt, tag=None):
            t = _t(singles, shape, dt, tag)
            spread_dma(t[:], ap_dram)
            return t

        # gather first: the h-chain is the critical startup path.
        # Critical-path loads go out immediately on near-empty queues; the
        # bulk weights are loaded later (emit_bulk_loads) in packed DMAs.
        gath = {}
        for blk in ("c", "r"):
            idx_sb = _t(sml, [L, 1], I32, "idx")
            nc.sync.dma_start(out=idx_sb[:], in_=x_idx_d[blk])
            xemb = _t(sml, [L, D], F32, "xemb")
            nc.gpsimd.indirect_dma_start(
                out=xemb[:], out_offset=None, in_=emb,
                in_offset=bass.IndirectOffsetOnAxis(ap=idx_sb[:, :1], axis=0))
            gath[blk] = xemb

        identf_sb = _t(singles, [L, L], F32, "idf")
        nc.sync.dma_start(out=identf_sb[:], in_=ident_f)
        identb_sb = _t(singles, [L, L], BF16, "idb")
        nc.sync.dma_start(out=identb_sb[:], in_=ident_b)
        Wh_sb = []
        W1_sb = []
        for k in range(2):
            t = _t(singles, [DC, D], F32, f"Wh{k}")
            nc.scalar.dma_start(out=t[:], in_=Wh[k * DC:(k + 1) * DC, :])
            Wh_sb.append(t)
        for k in range(2):
            t = _t(singles, [DC, D], F32, f"W1{k}")
            nc.scalar.dma_start(out=t[:], in_=W1[k * DC:(k + 1) * DC, :])
            W1_sb.append(t)
        brep_sb = _t(singles, [L, D], F32, "brep")
        nc.sync.dma_start(out=brep_sb[:], in_=b_rep)

        W2_sb = [_t(singles, [DC, D], F32, f"W2{k}") for k in range(2)]
        Wf1_sb = [_t(singles, [DC, D], BF16, f"Wg1{k}") for k in range(2)]
        Wf2_sb = [_t(singles, [DC, D], BF16, f"Wg2{k}") for k in range(2)]
        mask_sb = {"c": _t(singles, [L, 2 * L], BF16, "mskc"),
                   "r": _t(singles, [L, 2 * L], BF16, "mskr")}
        ws1t = _t(singles, [DC, 4, 2 * D], BF16, "ws1t")
        wst = _t(singles, [DC, 4, 2 * D], BF16, "wst")
        f1bf_sb = _t(singles, [DC, 16, D], BF16, "f1bf")
        Ws1_sb = [ws1t[:, k, :] for k in range(4)]
        Ws_sb = [wst[:, k, :] for k in range(4)]
        f2row_sb = _t(singles, [1, D], F32, "f2row")

        def emit_bulk_loads():
            for k in range(2):
                spread_dma(W2_sb[k][:], W2[k * DC:(k + 1) * DC, :])
            mask_q = {"c": nc.sync, "r": nc.gpsimd}
            for blk in ("c", "r"):
                mask_q[blk].dma_start(out=mask_sb[blk][:], in_=masks_d[blk])
            for k in range(2):
                spread_dma(Wf1_sb[k][:], Wf1[k * DC:(k + 1) * DC, :])
                spread_dma(Wf2_sb[k][:], Wf2[k * DC:(k + 1) * DC, :])
            spread_dma(ws1t[:], Ws1.rearrange("(k p) e -> p k e", p=DC))
            spread_dma(wst[:], Ws.rearrange("(k p) e -> p k e", p=DC))
            spread_dma(f1bf_sb[:], F1bf)
            spread_dma(f2row_sb[:], F2row)

        # zero-padded group-weight buffers (shared c/r: zero cols persist,
        # the diag is rewritten per group). Emitted after the gathers so the
        # memsets don't delay the gather on the gpsimd queue.
        mkp_shared = [_t(singles, [L, GQ, 32], BF16, f"mkp{i}")
                      for i in range(2)]
        for i in range(2):
            nc.gpsimd.memset(mkp_shared[i][:], 0.0)

        ones2_bf = _t(singles, [2, L], BF16, "ones2bf")
        nc.vector.memset(ones2_bf[:], 1.0)
        ones_bf = _t(singles, [L, L], BF16, "onesbf")
        nc.vector.memset(ones_bf[:], 1.0)

        cv_sb = {"c": _t(singles, [DC, 4], F32, "cv"),
                 "r": _t(singles, [DC, 4], F32, "rv")}


        def transpose_to(dst_ap, src_ap, n_par, n_free, tag="tp", extra=None):
            tp = _t(ps_mm, [128, 512], F32, "mm")
            nc.tensor.transpose(out=tp[0:n_free, 0:n_par], in_=src_ap,
                                identity=identf_sb[0:n_par, 0:n_par])
            nc.scalar.copy(dst_ap, tp[0:n_free, 0:n_par])
            if extra is not None:
                nc.scalar.copy(extra, tp[0:n_free, 0:n_par])

        def transpose_bf(dst_ap, src_ap, n_par, n_free):
            tp = _t(ps_mm, [128, 512], F32, "mm").bitcast(BF16)
            nc.tensor.transpose(out=tp[0:n_free, 0:n_par], in_=src_ap,
                                identity=identb_sb[0:n_par, 0:n_par])
            nc.scalar.copy(dst_ap, tp[0:n_free, 0:n_par])

        def transpose100(src_ap, n_par, n_free, tag):
            dst = _t(work, [n_free, n_par], F32, tag)
            transpose_to(dst[:], src_ap, n_par, n_free)
            return dst

        def elu_from_psum_bf(ps_ap, shape, tag):
            r = _t(work, shape, F32, "elur")
            nc.scalar.activation(r[:], ps_ap, AF.Relu)
            mn = _t(work, shape, F32, "elum")
            nc.vector.tensor_scalar_min(mn[:], ps_ap, 0.0)
            ex = _t(work, shape, F32, "elue")
            nc.scalar.activation(ex[:], mn[:], AF.Exp)
            o = _t(work, shape, BF16, tag + "_ob")
            nc.vector.scalar_tensor_tensor(o[:], r[:], -1.0, ex[:],
                                           op0=ALU.add, op1=ALU.add)
            return o

        def elu_from_psum(ps_ap, shape, tag):
            r = _t(work, shape, F32, "elur")
            nc.scalar.activation(r[:], ps_ap, AF.Relu)
            mn = _t(work, shape, F32, "elum")
            nc.vector.tensor_scalar_min(mn[:], ps_ap, 0.0)
            ex = _t(work, shape, F32, "elue")
            nc.scalar.activation(ex[:], mn[:], AF.Exp)
            o = _t(work, shape, F32, tag + "_o")
            nc.vector.scalar_tensor_tensor(o[:], r[:], -1.0, ex[:],
                                           op0=ALU.add, op1=ALU.add)
            return o

        def prep_block(blk, bulk_hook=None):
            # ---------- h = elu(x @ Wh) (gather already issued) ----------
            # ordered so the flathl (h1 hi/lo) path — the main-loop critical
            # input — completes first; h2/Hall follow.
            xemb = gath[blk]
            xembT = [transpose100(xemb[:, k * DC:(k + 1) * DC], L, DC, f"xT{k}")
                     for k in range(2)]
            hpre = _t(ps_mm, [128, 512], F32, "mm")
            for k in range(2):
                nc.tensor.matmul(out=hpre[:, 0:D], lhsT=xembT[k][:], rhs=Wh_sb[k][:],
                                 start=(k == 0), stop=(k == 1))
            h_sb = elu_from_psum(hpre[:, 0:D], [L, D], "h")

            hT = [transpose100(h_sb[:, k * DC:(k + 1) * DC], L, DC, f"hT{k}")
                  for k in range(2)]
            hTbf = []
            for k in range(2):
                t = _t(sml, [DC, L], BF16, f"hTbf{k}")
                nc.vector.tensor_copy(t[:], hT[k][:])
                hTbf.append(t)

            h1ps = _t(ps_mm, [128, 512], F32, "mm")
            for k in range(2):
                nc.tensor.matmul(out=h1ps[:, 0:D], lhsT=hT[k][:], rhs=W1_sb[k][:],
                                 start=(k == 0), stop=(k == 1))
            h1b = _t(sml, [L, D], F32, "h1b")
            nc.vector.tensor_add(h1b[:], h1ps[:, 0:D], brep_sb[:])
            # exact bf16 hi/lo pair of h1+b, flattened to [2, 25600] via
            # SBUF->SBUF DMA (partition-major read, single-row write)
            h1hi = _t(sml, [L, D], BF16, "h1hi")
            nc.vector.tensor_copy(h1hi[:], h1b[:])
            h1rem = _t(sml, [L, D], F32, "h1rem")
            nc.vector.tensor_sub(h1rem[:], h1b[:], h1hi[:])
            h1lo = _t(sml, [L, D], BF16, "h1lo")
            nc.vector.tensor_copy(h1lo[:], h1rem[:])
            HF = L * D // 2
            flA = _t(blockp, [2, HF], BF16, "flathlA", bufs=1)
            flB = _t(blockp, [2, HF], BF16, "flathlB", bufs=1)
            nc.sync.dma_start(out=flA[0:1, :], in_=h1hi[0:64, :])
            nc.gpsimd.dma_start(out=flA[1:2, :], in_=h1lo[0:64, :])
            nc.sync.dma_start(out=flB[0:1, :], in_=h1hi[64:128, :])
            nc.gpsimd.dma_start(out=flB[1:2, :], in_=h1lo[64:128, :])
            flathl = (flA, flB)

            if bulk_hook is not None:
                bulk_hook()

            h_bf = _t(sml, [L, D], BF16, "hbf")
            nc.vector.tensor_copy(h_bf[:], h_sb[:])

            # ---------- h2 = h @ W2 ----------
            h2ps = _t(ps_mm, [128, 512], F32, "mm")
            for k in range(2):
                nc.tensor.matmul(out=h2ps[:, 0:D], lhsT=hT[k][:], rhs=W2_sb[k][:],
                                 start=(k == 0), stop=(k == 1))
            h2_sb = _t(sml, [L, D], F32, "h2sb")
            nc.scalar.copy(h2_sb[:], h2ps[:, 0:D])

            # HallRep[m, d] = sum_m' h[m', d] broadcast to all partitions
            hallp = _t(ps_mm, [128, 512], F32, "mm")
            nc.tensor.matmul(out=hallp[:, 0:D], lhsT=ones_bf[:], rhs=h_bf[:],
                             start=True, stop=True)
            hallrep = _t(blockp, [L, D], F32, f"hallrep_{blk}", bufs=1)
            nc.scalar.copy(hallrep[:], hallp[:, 0:D])

            return dict(h_sb=h_sb, h_bf=h_bf, hT=hT, hTbf=hTbf, h2_sb=h2_sb,
                        flathl=flathl, hallrep=hallrep, mkp=mkp_shared)

        FQ = 2 * D  # 400

        def emit_zprod(blk, st_, ci):
            """hrep -> G -> tanh -> exp -> z*h for chunk ci; stores zzh."""
            msk = mask_sb[blk]
            h2_sb, h_bf, flathl = st_["h2_sb"], st_["h_bf"], st_["flathl"]
            mkp = st_["mkp"]

            # hrep: h1b (hi+lo) broadcast, two 2-bank tiles (4 queries each)
            G_sb = _t(zp, [L, CHUNK_I * D], F32, "G", bufs=2)
            HF = L * D // 2
            for half in range(2):
                hr = _t(ps_hrep, [L, 2, 512], F32, "hr")
                for k in range(2):
                    o = (ci * CHUNK_I + 4 * half + 2 * k) * D
                    fl = flathl[o // HF]
                    nc.tensor.matmul(out=hr[:, k, 0:FQ],
                                     lhsT=ones2_bf[:],
                                     rhs=fl[:, o % HF:o % HF + FQ],
                                     start=True, stop=True)
                nc.vector.tensor_add(
                    G_sb[:, half * 4 * D:(half + 1) * 4 * D].rearrange(
                        "p (k a d) -> p k a d", k=2, d=D),
                    hr[:, :, 0:FQ].rearrange("p k (a d) -> p k a d", d=D),
                    h2_sb[:].unsqueeze(1).unsqueeze(1).to_broadcast(
                        [L, 2, 2, D]))
            A_sb = _t(zp, [L, CHUNK_I * D], F32, "A", bufs=2)
            nc.scalar.activation(A_sb[:], G_sb[:], AF.Tanh, scale=1.0 / C_VAL)
            zzh = _t(zp, [L, 2, CHUNK_I, D], BF16, "zzh")
            nc.scalar.activation(
                zzh[:, 0, :, :],
                A_sb[:].rearrange("p (a d) -> p a d", d=D),
                AF.Exp, scale=C_VAL)
            zh_eng = nc.vector if ci % 2 == 0 else nc.gpsimd
            zh_eng.tensor_mul(
                zzh[:, 1, :, :], zzh[:, 0, :, :],
                h_bf[:].unsqueeze(1).to_broadcast([L, CHUNK_I, D]))
            st_.setdefault("zzh", {})[ci] = zzh

            # group weights: write the diag (int32 pair-copies; float engines
            # could flush the bf16-pair bit patterns as denormals). Two
            # iterations ahead of the group's ST MMs (SKEW=2-compatible).
            g = ci // 2
            if ci % 2 == 0:
                mk = st_["mkp"][g % 2]
                mkf = mk[:].rearrange("p a b -> p (a b)").bitcast(I32)
                nc.gpsimd.tensor_copy(
                    mkf[:, 0:(17 * (GQ - 1) + 1):17],
                    msk[:].bitcast(I32)[:, GQ * g:GQ * (g + 1)])
                st_[("mk", g)] = mk


        def emit_st(blk, st_, ci, st_tiles):
            """S/T accumulating matmuls for chunk ci (2-chunk skew)."""
            g = ci // 2
            mk = st_[("mk", g)]
            zzh = st_["zzh"].pop(ci)
            stt = st_tiles[g // 3]
            base = 32 * (g % 3)
            for jj in range(CHUNK_I):
                j = (ci % 2) * CHUNK_I + jj      # 0..15 within group
                nc.tensor.matmul(
                    out=stt[base:base + 32, 0:FQ],
                    lhsT=mk[:, j, :],
                    rhs=zzh[:, :, jj, :],
                    start=(j == 0), stop=(j == GQ - 1))

        def emit_post(blk, st_, ti, st_tiles, s_pk):
            """softmax post for ST tile ti: s = adjusted T/S -> s_pk[ti]."""
            hallrep = st_["hallrep"]
            nrows = 96 if ti < 2 else 64
            stt = st_tiles[ti]
            S = stt[0:nrows, 0:D]
            T = stt[0:nrows, D:2 * D]
            hall = hallrep[0:nrows, :]
            ind = _t(work, [nrows, D], F32, "ind")
            nc.vector.tensor_scalar(out=ind[:], in0=S, scalar1=0.0,
                                    scalar2=None, op0=ALU.is_equal)
            S1 = _t(work, [nrows, D], F32, "S1")
            nc.vector.scalar_tensor_tensor(S1[:], ind[:], 128.0, S,
                                           op0=ALU.mult, op1=ALU.add)
            Sinv = _t(work, [nrows, D], F32, "Sinv")
            nc.vector.reciprocal_approx_fast(out=Sinv[:], in_=S1[:])
            TH = _t(work, [nrows, D], F32, "TH")
            nc.gpsimd.tensor_mul(TH[:], ind[:], hall)
            T1 = _t(work, [nrows, D], F32, "T1")
            nc.vector.tensor_add(T1[:], T, TH[:])
            sp = _t(blockp, [nrows, D], F32, f"spk{ti}", bufs=1)
            nc.vector.tensor_mul(sp[:], T1[:], Sinv[:])
            s_pk[ti] = sp

        def emit_s_transpose(blk, st_, ti, s_pk, sT_all, sTbf_all):
            nrows = 96 if ti < 2 else 64
            sp = s_pk[ti]
            for ch in range(2):
                transpose_to(sT_all[ch][:, 96 * ti:96 * ti + nrows],
                             sp[:, ch * DC:(ch + 1) * DC], nrows, DC,
                             extra=sTbf_all[ch][:, 96 * ti:96 * ti + nrows])

        def gate_tile_piece(blk, st_, sT_all, sTbf_all, fT_all, ti, dire):
            """fps -> tsig -> fT for one ST tile and direction (runs while
            the main loop continues; only needs that tile's sT columns)."""
            hTbf = st_["hTbf"]
            nq = 48 if ti < 2 else 32
            c0 = 96 * ti
            fps = _t(ps_mm, [128, 512], F32, "mm")
            for ch in range(2):
                nc.tensor.matmul(out=fps[0:nq, 0:D],
                                 lhsT=sTbf_all[ch][:, c0 + dire:c0 + 2 * nq:2],
                                 rhs=Wf1_sb[ch][:],
                                 start=(ch == 0), stop=False)
            for ch in range(2):
                nc.tensor.matmul(out=fps[0:nq, 0:D],
                                 lhsT=hTbf[ch][:, 48 * ti:48 * ti + nq],
                                 rhs=Wf2_sb[ch][:],
                                 start=False, stop=(ch == 1))
            tsig = _t(work, [48, D], BF16, "tsig")
            nc.scalar.activation(tsig[0:nq, :], fps[0:nq, 0:D],
                                 AF.Tanh, scale=0.5)
            for ch in range(2):
                transpose_bf(fT_all[dire][ch][:, 48 * ti:48 * ti + nq],
                             tsig[0:nq, ch * DC:(ch + 1) * DC], nq, DC)

        def gate_final_pieces(blk, st_, sT_all, fT_all):
            """u from fT_all; att_s; cv — after all three tiles are in."""
            hT = st_["hT"]
            gs = {"uT": {}}

            def u_piece(dire):
                for ch in range(2):
                    fT = fT_all[dire][ch]
                    nc.vector.tensor_scalar(out=fT[:], in0=fT[:], scalar1=0.5,
                                            scalar2=0.5, op0=ALU.mult,
                                            op1=ALU.add)
                    sTv = sT_all[ch][:, dire::2]
                    dt_ = _t(work, [DC, L], F32, f"d{dire}{ch}")
                    nc.vector.tensor_sub(dt_[:], hT[ch][:], sTv)
                    nc.vector.tensor_mul(dt_[:], fT[:], dt_[:])
                    u = _t(blockp, [DC, L], F32, f"uT{blk}{dire}{ch}", bufs=1)
                    nc.vector.tensor_add(u[:], sTv, dt_[:])
                    ub = _t(blockp, [DC, L], BF16, f"uTb{blk}{dire}{ch}",
                            bufs=1)
                    nc.vector.tensor_copy(ub[:], u[:])
                    gs["uT"][(dire, ch)] = u
                    gs.setdefault("uTb", {})[(dire, ch)] = ub

            def ws1_piece():
                gs["uT_list"] = [gs["uT"][(0, 0)], gs["uT"][(0, 1)],
                                 gs["uT"][(1, 0)], gs["uT"][(1, 1)]]
                uTb_list = [gs["uTb"][(0, 0)], gs["uTb"][(0, 1)],
                            gs["uTb"][(1, 0)], gs["uTb"][(1, 1)]]
                wps = _t(ps_mm, [128, 512], F32, "mm")
                for q in range(4):
                    nc.tensor.matmul(out=wps[:, 0:2 * D], lhsT=uTb_list[q][:],
                                     rhs=Ws1_sb[q][:],
                                     start=(q == 0), stop=(q == 3))
                gs["w_sb"] = elu_from_psum_bf(wps[:, 0:2 * D], [L, 2 * D],
                                              "w")

            def wt_piece(q0):
                for q in (q0, q0 + 1):
                    dst = _t(work, [DC, L], BF16, f"wT{q}")
                    transpose_bf(dst[:], gs["w_sb"][:, q * DC:(q + 1) * DC],
                                 L, DC)
                    gs.setdefault("wT", {})[q] = dst

            def ws_piece():
                aps = _t(ps_mm, [128, 512], F32, "mm")
                for q in range(4):
                    nc.tensor.matmul(out=aps[:, 0:2 * D], lhsT=gs["wT"][q][:],
                                     rhs=Ws_sb[q][:],
                                     start=(q == 0), stop=(q == 3))
                atts_sb = _t(work, [L, 2 * D], F32, "atts")
                nc.scalar.copy(atts_sb[:], aps[:, 0:2 * D])
                gs["atts"] = atts_sb

            def cv_piece(q0):
                for q in (q0, q0 + 1):
                    aT = _t(ps_mm, [128, 512], F32, "mm")
                    nc.tensor.transpose(out=aT[0:DC, 0:L],
                                        in_=gs["atts"][:, q * DC:(q + 1) * DC],
                                        identity=identf_sb[:, :])
                    vT = _t(work, [DC, L], F32, "vT")
                    nc.vector.scalar_tensor_tensor(
                        vT[:], gs["uT_list"][q][:], 1.0, aT[0:DC, 0:L],
                        op0=ALU.mult, op1=ALU.mult,
                        accum_out=cv_sb[blk][:, q:q + 1])

            return [lambda: u_piece(0), lambda: u_piece(1),
                    ws1_piece, lambda: wt_piece(0), lambda: wt_piece(2),
                    ws_piece, lambda: cv_piece(0), lambda: cv_piece(2)]

        # ================== emission schedule ==================
        st_c = prep_block("c", bulk_hook=emit_bulk_loads)

        st_tiles = {}
        s_pk = {"c": {}, "r": {}}
        sT_all = {"c": [_t(blockp, [DC, 2 * L], F32, f"sTc{ch}", bufs=1)
                        for ch in range(2)],
                  "r": [_t(blockp, [DC, 2 * L], F32, f"sTr{ch}", bufs=1)
                        for ch in range(2)]}
        fT_all = {b: [[_t(blockp, [DC, L], BF16, f"fT{b}{dd}{ch}", bufs=1)
                       for ch in range(2)] for dd in range(2)]
                  for b in ("c", "r")}
        sTbf_all = {b: [_t(blockp, [DC, 2 * L], BF16, f"sTb{b}{ch}", bufs=1)
                        for ch in range(2)]
                    for b in ("c", "r")}

        # global chunk order: c0..c15, r0..r15; ST matmuls trail z-production
        # by SKEW chunks so the tensor queue always has ready work ahead of
        # the scalar/vector z chain.
        SKEW = 2
        seq = [("c", i) for i in range(N_CHUNKS)] + \
              [("r", i) for i in range(N_CHUNKS)]
        sts = {"c": st_c}

        def tiles_for(blk):
            if blk not in st_tiles:
                st_tiles[blk] = [_t(ps_st, [128, 512], F32, "st")
                                 for _ in range(3)]
            return st_tiles[blk]

        pending_gate = []
        for it in range(len(seq) + SKEW):
            if it == 2:
                # r prep interleaves with c's early main loop
                sts["r"] = prep_block("r")
            if it < len(seq):
                blk, ci = seq[it]
                emit_zprod(blk, sts[blk], ci)
            if it >= SKEW:
                blk, ci = seq[it - SKEW]
                st_ = sts[blk]
                emit_st(blk, st_, ci, tiles_for(blk))
                # tile boundary actions keyed on the just-emitted ST chunk
                if ci in (5, 11, 15):
                    ti = (5, 11, 15).index(ci)
                    emit_post(blk, st_, ti, st_tiles[blk], s_pk[blk])
                if ci in (6, 12):
                    ti = (6, 12).index(ci)
                    emit_s_transpose(blk, st_, ti, s_pk[blk], sT_all[blk],
                                     sTbf_all[blk])
                    pending_gate.append(
                        lambda b=blk, t=ti: gate_tile_piece(
                            b, sts[b], sT_all[b], sTbf_all[b],
                            fT_all[b], t, 0))
                    pending_gate.append(
                        lambda b=blk, t=ti: gate_tile_piece(
                            b, sts[b], sT_all[b], sTbf_all[b],
                            fT_all[b], t, 1))
                if ci == 15:
                    emit_s_transpose(blk, st_, 2, s_pk[blk], sT_all[blk],
                                     sTbf_all[blk])
                    pending_gate.append(
                        lambda b=blk: gate_tile_piece(
                            b, sts[b], sT_all[b], sTbf_all[b],
                            fT_all[b], 2, 0))
                    pending_gate.append(
                        lambda b=blk: gate_tile_piece(
                            b, sts[b], sT_all[b], sTbf_all[b],
                            fT_all[b], 2, 1))
                    pending_gate.extend(gate_final_pieces(
                        blk, sts[blk], sT_all[blk], fT_all[blk]))
                elif pending_gate:
                    pending_gate.pop(0)()
        while pending_gate:
            pending_gate.pop(0)()

        # ---------- head: feat = [cv, rv, cv-rv, cv*rv]; y ----------
        # feat columns as exact bf16 hi/lo weight pairs x bf16 F1, one
        # [2, 200] psum accumulation; then y = sum(relu(y1) * F2row).
        diff = _t(singles, [DC, 4], F32, "diff")
        nc.vector.tensor_sub(diff[:], cv_sb["c"][:], cv_sb["r"][:])
        prod = _t(singles, [DC, 4], F32, "prod")
        nc.vector.tensor_mul(prod[:], cv_sb["c"][:], cv_sb["r"][:])
        groups = [cv_sb["c"], cv_sb["r"], diff, prod]

        featp = _t(singles, [DC, 4, 4, 2], BF16, "featp")
        for gi, grp in enumerate(groups):
            nc.vector.tensor_copy(featp[:, gi, :, 0], grp[:])
            rem = _t(sml, [DC, 4], F32, "rem")
            nc.vector.tensor_sub(rem[:], grp[:], featp[:, gi, :, 0])
            nc.vector.tensor_copy(featp[:, gi, :, 1], rem[:])

        y1p = _t(ps_mm, [128, 512], F32, "mm")
        for kc in range(16):
            nc.tensor.matmul(out=y1p[0:2, 0:D],
                             lhsT=featp[:, kc // 4, kc % 4, :],
                             rhs=f1bf_sb[:, kc, :],
                             start=(kc == 0), stop=(kc == 15))
        y2sb = _t(sml, [2, D], F32, "y2sb")
        nc.scalar.copy(y2sb[:], y1p[0:2, 0:D])
        onesf2 = _t(sml, [2, 1], F32, "onesf2")
        nc.vector.memset(onesf2[:], 1.0)
        yrow = _t(ps_mm, [128, 512], F32, "mm")
        nc.tensor.matmul(out=yrow[0:1, 0:D], lhsT=onesf2[:], rhs=y2sb[:],
                         start=True, stop=True)
        r1 = _t(sml, [1, D], F32, "r1")
        nc.scalar.activation(r1[:], yrow[0:1, 0:D], AF.Relu)
        ym = _t(sml, [1, D], F32, "ym")
        nc.vector.tensor_mul(ym[:], r1[:], f2row_sb[:])
        y_sb = _t(sml, [1, 1], F32, "ysb")
        nc.vector.tensor_reduce(out=y_sb[:], in_=ym[:], axis=AX.X, op=ALU.add)
        nc.sync.dma_start(out=y_out, in_=y_sb[:])

    nc.compile()
    return nc


def _build_masks(ids):
    """[128, 256] bf16: col 2i+0 = fw col for query i (keys m>i), 2i+1 = bw
    (m<i); pad keys and pad queries zero the column."""
    np1 = (ids != PAD).astype(np.float32)
    m = np.arange(L)
    fw = (m[:, None] > m[None, :]).astype(np.float32) * np1[:, None] * np1[None, :]
    bw = (m[:, None] < m[None, :]).astype(np.float32) * np1[:, None] * np1[None, :]
    out = np.empty((L, 2 * L), np.float32)
    out[:, 0::2] = fw
    out[:, 1::2] = bw
    return out.astype(ml_dtypes.bfloat16)


def make_in_maps(inputs):
    x1 = np.asarray(inputs["x1"]).astype(np.int64)
    x2 = np.asarray(inputs["x2"]).astype(np.int64)
    f32 = lambda k: np.ascontiguousarray(np.asarray(inputs[k], np.float32))
    emb = f32("emb_w")
    shared = {
        "emb": emb,
        "Wh": f32("Wh_w"), "W1": f32("W1_w"), "W2": f32("W2_w"),
        "Wf1": f32("Wf1_w").astype(ml_dtypes.bfloat16),
        "Wf2": f32("Wf2_w").astype(ml_dtypes.bfloat16),
        "Ws1": f32("Ws1_w").astype(ml_dtypes.bfloat16),
        "Ws": f32("Ws_w").astype(ml_dtypes.bfloat16),
        "F1bf": np.ascontiguousarray(
            f32("F1_w").reshape(16, DC, D).transpose(1, 0, 2)
        ).astype(ml_dtypes.bfloat16),
        "F2row": f32("F2_w").reshape(1, D),
        "b_rep": np.tile(f32("b").reshape(1, D), (L, 1)),
        "ident_f": np.eye(L, dtype=np.float32),
        "ident_b": np.eye(L, dtype=np.float32).astype(ml_dtypes.bfloat16),
    }
    in_maps = []
    for bidx in range(N_CORES):
        m = dict(shared)
        m["xc_idx"] = x1[bidx].reshape(L, 1).astype(np.int32)
        m["xr_idx"] = x2[bidx].reshape(L, 1).astype(np.int32)
        m["masks_c"] = _build_masks(x1[bidx])
        m["masks_r"] = _build_masks(x2[bidx])
        in_maps.append(m)
    return in_maps


_NC_CACHE = {}


def get_nc():
    if "nc" not in _NC_CACHE:
        _NC_CACHE["nc"] = build_nc()
    return _NC_CACHE["nc"]


def kernel(**inputs) -> np.ndarray:
    from concourse.bass_utils import run_bass_kernel_spmd
    nc = get_nc()
    in_maps = make_in_maps(inputs)
    res = run_bass_kernel_spmd(nc, in_maps, list(range(N_CORES)))
    y = np.array([np.asarray(res.results[i]["y"]).reshape(-1)[0]
                  for i in range(N_CORES)], dtype=np.float32)
    return y


# revision 5
# speedup vs baseline: 1.1960x; 1.1960x over previous
"""DiSAN Trainium2 Bass kernel v4 — 8-core data parallel (one example per core).

S/T (directional-softmax denominator and h-weighted numerator) via
zero-padded mask WEIGHTS + PSUM accumulation: a group of 16 queries shares
one [m=128, 32] weight matrix (query j's fw/bw mask pair in columns 2j:2j+2,
zeros elsewhere, built on-device by an int32 stride-17 diag copy); 16
matmuls — one per query, each streaming that query's [z | z*h] 400 columns —
accumulate into one [32, 400] PSUM region. Zero-weight columns add exact
zeros, so the region ends up densely packed [2q+dir, (S|T)] with no
extraction. Groups stack at partition bases {0, 32, 64} of a bank, giving
[96, 400] tiles for vectorized softmax post (s = T/S with the all-masked
uniform fallback, reciprocal_approx_fast). This replaces v1's 4
LDWEIGHTS+MATMUL pairs per query with 1, ~3.5x less tensor time.

Pipeline: z-production (hrep broadcast MMs -> G add -> tanh -> exp -> z*h)
runs 2 chunks ahead of the S/T matmuls (SKEW) so the tensor queue always has
ready work; z*h alternates Vector/GpSimd. The packed s is PE-transposed into
sT (fp32 + bf16 copies); the fusion gate runs per-ST-tile in bf16
(Wf1/Wf2/Ws1/Ws and all gate transposes bf16) so most of it overlaps the
other block's main loop. Head: feat columns as exact bf16 hi/lo weight pairs
against bf16 F1 into one [2, 200] accumulation, then y = sum(relu(y1)*F2row).

kernel(**inputs) takes the full unsharded inputs and returns the full (8,)
output; batch is sharded across the 8 NeuronCores via run_bass_kernel_spmd.
"""

from contextlib import ExitStack

import numpy as np
import ml_dtypes

import concourse.bass as bass
import concourse.bacc as bacc
import concourse.tile as tile
from concourse import mybir

F32 = mybir.dt.float32
BF16 = mybir.dt.bfloat16
I32 = mybir.dt.int32
AF = mybir.ActivationFunctionType
ALU = mybir.AluOpType
AX = mybir.AxisListType

L = 128          # sequence length
D = 200          # feature dim
DC = 100         # feature chunk (2 chunks of 100)
VOCAB = 32000
PAD = 1
N_CORES = 8
CHUNK_I = 8      # queries per G/z chunk
N_CHUNKS = L // CHUNK_I   # 16
GQ = 16          # queries per S/T accumulation group (2 chunks)
C_VAL = 5.0


def build_nc():
    nc = bacc.Bacc("TRN2", target_bir_lowering=False, debug=False)

    def din(name, shape, dt):
        return nc.dram_tensor(name, shape, dt, kind="ExternalInput").ap()

    x_idx_d = {"c": din("xc_idx", [L, 1], I32), "r": din("xr_idx", [L, 1], I32)}
    emb = din("emb", [VOCAB, D], F32)
    Wh = din("Wh", [D, D], F32)
    W1 = din("W1", [D, D], F32)
    W2 = din("W2", [D, D], F32)
    Wf1 = din("Wf1", [D, D], BF16)
    Wf2 = din("Wf2", [D, D], BF16)
    Ws1 = din("Ws1", [2 * D, 2 * D], BF16)
    Ws = din("Ws", [2 * D, 2 * D], BF16)
    F1bf = din("F1bf", [DC, 16, D], BF16)
    F2row = din("F2row", [1, D], F32)
    b_rep = din("b_rep", [L, D], F32)
    masks_d = {"c": din("masks_c", [L, 2 * L], BF16),
               "r": din("masks_r", [L, 2 * L], BF16)}
    ident_f = din("ident_f", [L, L], F32)
    ident_b = din("ident_b", [L, L], BF16)

    y_out = nc.dram_tensor("y", [1, 1], F32, kind="ExternalOutput").ap()

    # SBUF->SBUF flatten needs no DRAM scratch
    with tile.TileContext(nc) as tc, ExitStack() as ctx:
        singles = ctx.enter_context(tc.tile_pool(name="singles", bufs=1))
        blockp = ctx.enter_context(tc.tile_pool(name="blockp", bufs=2))
        work = ctx.enter_context(tc.tile_pool(name="work", bufs=2))
        sml = ctx.enter_context(tc.tile_pool(name="sml", bufs=2))
        zp = ctx.enter_context(tc.tile_pool(name="zp", bufs=4))
        ps_hrep = ctx.enter_context(tc.tile_pool(name="ps_hrep", bufs=2, space="PSUM"))
        ps_st = ctx.enter_context(tc.tile_pool(name="ps_st", bufs=2, space="PSUM"))
        ps_mm = ctx.enter_context(tc.tile_pool(name="ps_mm", bufs=2, space="PSUM"))

        def _t(pool, shape, dt, tag, **kw):
            return pool.tile(shape, dt, name=tag, tag=tag, **kw)

        _dmaq = [nc.sync, nc.scalar, nc.gpsimd]
        _dmaqi = [0]

        def spread_dma(out, in_):
            eng = _dmaq[_dmaqi[0] % len(_dmaq)]
            _dmaqi[0] += 1
            eng.dma_start(out=out, in_=in_)

        def load(ap_dram, shape, dt, tag=None):
            t = _t(singles, shape, dt, tag)
            spread_dma(t[:], ap_dram)
            return t

        # gather first: the h-chain is the critical startup path.
        # Critical-path loads go out immediately on near-empty queues; the
        # bulk weights are loaded later (emit_bulk_loads) in packed DMAs.
        gath = {}
        for blk in ("c", "r"):
            idx_sb = _t(sml, [L, 1], I32, "idx")
            nc.sync.dma_start(out=idx_sb[:], in_=x_idx_d[blk])
            xemb = _t(sml, [L, D], F32, "xemb")
            nc.gpsimd.indirect_dma_start(
                out=xemb[:], out_offset=None, in_=emb,
                in_offset=bass.IndirectOffsetOnAxis(ap=idx_sb[:, :1], axis=0))
            gath[blk] = xemb

        identf_sb = _t(singles, [L, L], F32, "idf")
        nc.sync.dma_start(out=identf_sb[:], in_=ident_f)
        identb_sb = _t(singles, [L, L], BF16, "idb")
        nc.sync.dma_start(out=identb_sb[:], in_=ident_b)
        Wh_sb = []
        W1_sb = []
        for k in range(2):
            t = _t(singles, [DC, D], F32, f"Wh{k}")
            nc.scalar.dma_start(out=t[:], in_=Wh[k * DC:(k + 1) * DC, :])
            Wh_sb.append(t)
        for k in range(2):
            t = _t(singles, [DC, D], F32, f"W1{k}")
            nc.scalar.dma_start(out=t[:], in_=W1[k * DC:(k + 1) * DC, :])
            W1_sb.append(t)
        brep_sb = _t(singles, [L, D], F32, "brep")
        nc.sync.dma_start(out=brep_sb[:], in_=b_rep)

        W2_sb = [_t(singles, [DC, D], F32, f"W2{k}") for k in range(2)]
        Wf1_sb = [_t(singles, [DC, D], BF16, f"Wg1{k}") for k in range(2)]
        Wf2_sb = [_t(singles, [DC, D], BF16, f"Wg2{k}") for k in range(2)]
        mask_sb = {"c": _t(singles, [L, 2 * L], BF16, "mskc"),
                   "r": _t(singles, [L, 2 * L], BF16, "mskr")}
        ws1t = _t(singles, [DC, 4, 2 * D], BF16, "ws1t")
        wst = _t(singles, [DC, 4, 2 * D], BF16, "wst")
        f1bf_sb = _t(singles, [DC, 16, D], BF16, "f1bf")
        Ws1_sb = [ws1t[:, k, :] for k in range(4)]
        Ws_sb = [wst[:, k, :] for k in range(4)]
        f2row_sb = _t(singles, [1, D], F32, "f2row")

        def emit_bulk_loads():
            for k in range(2):
                spread_dma(W2_sb[k][:], W2[k * DC:(k + 1) * DC, :])
            mask_q = {"c": nc.sync, "r": nc.gpsimd}
            for blk in ("c", "r"):
                mask_q[blk].dma_start(out=mask_sb[blk][:], in_=masks_d[blk])
            for k in range(2):
                spread_dma(Wf1_sb[k][:], Wf1[k * DC:(k + 1) * DC, :])
                spread_dma(Wf2_sb[k][:], Wf2[k * DC:(k + 1) * DC, :])
            spread_dma(ws1t[:], Ws1.rearrange("(k p) e -> p k e", p=DC))
            spread_dma(wst[:], Ws.rearrange("(k p) e -> p k e", p=DC))
            spread_dma(f1bf_sb[:], F1bf)
            spread_dma(f2row_sb[:], F2row)

        # zero-padded group-weight buffers (shared c/r: zero cols persist,
        # the diag is rewritten per group). Emitted after the gathers so the
        # memsets don't delay the gather on the gpsimd queue.
        mkp_shared = [_t(singles, [L, GQ, 32], BF16, f"mkp{i}")
                      for i in range(2)]
        for i in range(2):
            nc.gpsimd.memset(mkp_shared[i][:], 0.0)

        ones2_bf = _t(singles, [2, L], BF16, "ones2bf")
        nc.vector.memset(ones2_bf[:], 1.0)
        ones_bf = _t(singles, [L, L], BF16, "onesbf")
        nc.vector.memset(ones_bf[:], 1.0)

        cv_sb = {"c": _t(singles, [DC, 4], F32, "cv"),
                 "r": _t(singles, [DC, 4], F32, "rv")}


        def transpose_to(dst_ap, src_ap, n_par, n_free, tag="tp", extra=None):
            tp = _t(ps_mm, [128, 512], F32, "mm")
            nc.tensor.transpose(out=tp[0:n_free, 0:n_par], in_=src_ap,
                                identity=identf_sb[0:n_par, 0:n_par])
            nc.scalar.copy(dst_ap, tp[0:n_free, 0:n_par])
            if extra is not None:
                nc.scalar.copy(extra, tp[0:n_free, 0:n_par])

        def transpose_bf(dst_ap, src_ap, n_par, n_free):
            tp = _t(ps_mm, [128, 512], F32, "mm").bitcast(BF16)
            nc.tensor.transpose(out=tp[0:n_free, 0:n_par], in_=src_ap,
                                identity=identb_sb[0:n_par, 0:n_par])
            nc.scalar.copy(dst_ap, tp[0:n_free, 0:n_par])

        def transpose100(src_ap, n_par, n_free, tag):
            dst = _t(work, [n_free, n_par], F32, tag)
            transpose_to(dst[:], src_ap, n_par, n_free)
            return dst

        def elu_from_psum_bf(ps_ap, shape, tag):
            r = _t(work, shape, F32, "elur")
            nc.scalar.activation(r[:], ps_ap, AF.Relu)
            mn = _t(work, shape, F32, "elum")
            nc.vector.tensor_scalar_min(mn[:], ps_ap, 0.0)
            ex = _t(work, shape, F32, "elue")
            nc.scalar.activation(ex[:], mn[:], AF.Exp)
            o = _t(work, shape, BF16, tag + "_ob")
            nc.vector.scalar_tensor_tensor(o[:], r[:], -1.0, ex[:],
                                           op0=ALU.add, op1=ALU.add)
            return o

        def elu_from_psum(ps_ap, shape, tag):
            r = _t(work, shape, F32, "elur")
            nc.scalar.activation(r[:], ps_ap, AF.Relu)
            mn = _t(work, shape, F32, "elum")
            nc.vector.tensor_scalar_min(mn[:], ps_ap, 0.0)
            ex = _t(work, shape, F32, "elue")
            nc.scalar.activation(ex[:], mn[:], AF.Exp)
            o = _t(work, shape, F32, tag + "_o")
            nc.vector.scalar_tensor_tensor(o[:], r[:], -1.0, ex[:],
                                           op0=ALU.add, op1=ALU.add)
            return o

        def prep_block(blk, bulk_hook=None):
            # ---------- h = elu(x @ Wh) (gather already issued) ----------
            # ordered so the flathl (h1 hi/lo) path — the main-loop critical
            # input — completes first; h2/Hall follow.
            xemb = gath[blk]
            xembT = [transpose100(xemb[:, k * DC:(k + 1) * DC], L, DC, f"xT{k}")
                     for k in range(2)]
            hpre = _t(ps_mm, [128, 512], F32, "mm")
            for k in range(2):
                nc.tensor.matmul(out=hpre[:, 0:D], lhsT=xembT[k][:], rhs=Wh_sb[k][:],
                                 start=(k == 0), stop=(k == 1))
            h_sb = elu_from_psum(hpre[:, 0:D], [L, D], "h")

            hT = [transpose100(h_sb[:, k * DC:(k + 1) * DC], L, DC, f"hT{k}")
                  for k in range(2)]
            hTbf = []
            for k in range(2):
                t = _t(sml, [DC, L], BF16, f"hTbf{k}")
                nc.vector.tensor_copy(t[:], hT[k][:])
                hTbf.append(t)

            h1ps = _t(ps_mm, [128, 512], F32, "mm")
            for k in range(2):
                nc.tensor.matmul(out=h1ps[:, 0:D], lhsT=hT[k][:], rhs=W1_sb[k][:],
                                 start=(k == 0), stop=(k == 1))
            h1b = _t(sml, [L, D], F32, "h1b")
            nc.vector.tensor_add(h1b[:], h1ps[:, 0:D], brep_sb[:])
            # exact bf16 hi/lo pair of h1+b, flattened to [2, 25600] via
            # SBUF->SBUF DMA (partition-major read, single-row write)
            h1hi = _t(sml, [L, D], BF16, "h1hi")
            nc.vector.tensor_copy(h1hi[:], h1b[:])
            h1rem = _t(sml, [L, D], F32, "h1rem")
            nc.vector.tensor_sub(h1rem[:], h1b[:], h1hi[:])
            h1lo = _t(sml, [L, D], BF16, "h1lo")
            nc.vector.tensor_copy(h1lo[:], h1rem[:])
            HF = L * D // 2
            flA = _t(blockp, [2, HF], BF16, "flathlA", bufs=1)
            flB = _t(blockp, [2, HF], BF16, "flathlB", bufs=1)
            nc.sync.dma_start(out=flA[0:1, :], in_=h1hi[0:64, :])
            nc.gpsimd.dma_start(out=flA[1:2, :], in_=h1lo[0:64, :])
            nc.sync.dma_start(out=flB[0:1, :], in_=h1hi[64:128, :])
            nc.gpsimd.dma_start(out=flB[1:2, :], in_=h1lo[64:128, :])
            flathl = (flA, flB)

            if bulk_hook is not None:
                bulk_hook()

            h_bf = _t(sml, [L, D], BF16, "hbf")
            nc.vector.tensor_copy(h_bf[:], h_sb[:])

            # ---------- h2 = h @ W2 ----------
            h2ps = _t(ps_mm, [128, 512], F32, "mm")
            for k in range(2):
                nc.tensor.matmul(out=h2ps[:, 0:D], lhsT=hT[k][:], rhs=W2_sb[k][:],
                                 start=(k == 0), stop=(k == 1))
            h2_sb = _t(sml, [L, D], F32, "h2sb")
            nc.scalar.copy(h2_sb[:], h2ps[:, 0:D])

            # HallRep[m, d] = sum_m' h[m', d] broadcast to all partitions
            hallp = _t(ps_mm, [128, 512], F32, "mm")
            nc.tensor.matmul(out=hallp[:, 0:D], lhsT=ones_bf[:], rhs=h_bf[:],
                             start=True, stop=True)
            hallrep = _t(blockp, [L, D], F32, f"hallrep_{blk}", bufs=1)
            nc.scalar.copy(hallrep[:], hallp[:, 0:D])

            return dict(h_sb=h_sb, h_bf=h_bf, hT=hT, hTbf=hTbf, h2_sb=h2_sb,
                        flathl=flathl, hallrep=hallrep, mkp=mkp_shared)

        FQ = 2 * D  # 400

        def emit_zprod(blk, st_, ci):
            """hrep -> G -> tanh -> exp -> z*h for chunk ci; stores zzh."""
            msk = mask_sb[blk]
            h2_sb, h_bf, flathl = st_["h2_sb"], st_["h_bf"], st_["flathl"]
            mkp = st_["mkp"]

            # hrep: h1b (hi+lo) broadcast, two 2-bank tiles (4 queries each)
            G_sb = _t(zp, [L, CHUNK_I * D], F32, "G", bufs=2)
            HF = L * D // 2
            for half in range(2):
                hr = _t(ps_hrep, [L, 2, 512], F32, "hr")
                for k in range(2):
                    o = (ci * CHUNK_I + 4 * half + 2 * k) * D
                    fl = flathl[o // HF]
                    nc.tensor.matmul(out=hr[:, k, 0:FQ],
                                     lhsT=ones2_bf[:],
                                     rhs=fl[:, o % HF:o % HF + FQ],
                                     start=True, stop=True)
                nc.vector.tensor_add(
                    G_sb[:, half * 4 * D:(half + 1) * 4 * D].rearrange(
                        "p (k a d) -> p k a d", k=2, d=D),
                    hr[:, :, 0:FQ].rearrange("p k (a d) -> p k a d", d=D),
                    h2_sb[:].unsqueeze(1).unsqueeze(1).to_broadcast(
                        [L, 2, 2, D]))
            A_sb = _t(zp, [L, CHUNK_I * D], F32, "A", bufs=2)
            nc.scalar.activation(A_sb[:], G_sb[:], AF.Tanh, scale=1.0 / C_VAL)
            zzh = _t(zp, [L, 2, CHUNK_I, D], BF16, "zzh")
            nc.scalar.activation(
                zzh[:, 0, :, :],
                A_sb[:].rearrange("p (a d) -> p a d", d=D),
                AF.Exp, scale=C_VAL)
            zh_eng = nc.vector if ci % 2 == 0 else nc.gpsimd
            zh_eng.tensor_mul(
                zzh[:, 1, :, :], zzh[:, 0, :, :],
                h_bf[:].unsqueeze(1).to_broadcast([L, CHUNK_I, D]))
            st_.setdefault("zzh", {})[ci] = zzh

            # group weights: write the diag (int32 pair-copies; float engines
            # could flush the bf16-pair bit patterns as denormals). Two
            # iterations ahead of the group's ST MMs (SKEW=2-compatible).
            g = ci // 2
            if ci % 2 == 0:
                mk = st_["mkp"][g % 2]
                mkf = mk[:].rearrange("p a b -> p (a b)").bitcast(I32)
                nc.gpsimd.tensor_copy(
                    mkf[:, 0:(17 * (GQ - 1) + 1):17],
                    msk[:].bitcast(I32)[:, GQ * g:GQ * (g + 1)])
                st_[("mk", g)] = mk


        def emit_st(blk, st_, ci, st_tiles):
            """S/T accumulating matmuls for chunk ci (2-chunk skew)."""
            g = ci // 2
            mk = st_[("mk", g)]
            zzh = st_["zzh"].pop(ci)
            stt = st_tiles[g // 3]
            base = 32 * (g % 3)
            for jj in range(CHUNK_I):
                j = (ci % 2) * CHUNK_I + jj      # 0..15 within group
                nc.tensor.matmul(
                    out=stt[base:base + 32, 0:FQ],
                    lhsT=mk[:, j, :],
                    rhs=zzh[:, :, jj, :],
                    start=(j == 0), stop=(j == GQ - 1))

        def emit_post(blk, st_, ti, st_tiles, s_pk):
            """softmax post for ST tile ti: s = adjusted T/S -> s_pk[ti]."""
            hallrep = st_["hallrep"]
            nrows = 96 if ti < 2 else 64
            stt = st_tiles[ti]
            S = stt[0:nrows, 0:D]
            T = stt[0:nrows, D:2 * D]
            hall = hallrep[0:nrows, :]
            ind = _t(work, [nrows, D], F32, "ind")
            nc.vector.tensor_scalar(out=ind[:], in0=S, scalar1=0.0,
                                    scalar2=None, op0=ALU.is_equal)
            S1 = _t(work, [nrows, D], F32, "S1")
            nc.vector.scalar_tensor_tensor(S1[:], ind[:], 128.0, S,
                                           op0=ALU.mult, op1=ALU.add)
            Sinv = _t(work, [nrows, D], F32, "Sinv")
            nc.vector.reciprocal_approx_fast(out=Sinv[:], in_=S1[:])
            TH = _t(work, [nrows, D], F32, "TH")
            nc.gpsimd.tensor_mul(TH[:], ind[:], hall)
            T1 = _t(work, [nrows, D], F32, "T1")
            nc.vector.tensor_add(T1[:], T, TH[:])
            sp = _t(blockp, [nrows, D], F32, f"spk{ti}", bufs=1)
            nc.vector.tensor_mul(sp[:], T1[:], Sinv[:])
            s_pk[ti] = sp

        def emit_s_transpose(blk, st_, ti, s_pk, sT_all, sTbf_all):
            nrows = 96 if ti < 2 else 64
            sp = s_pk[ti]
            for ch in range(2):
                transpose_to(sT_all[ch][:, 96 * ti:96 * ti + nrows],
                             sp[:, ch * DC:(ch + 1) * DC], nrows, DC,
                             extra=sTbf_all[ch][:, 96 * ti:96 * ti + nrows])

        def gate_tile_piece(blk, st_, sT_all, sTbf_all, fT_all, ti, dire):
            """fps -> tsig -> fT for one ST tile and direction (runs while
            the main loop continues; only needs that tile's sT columns)."""
            hTbf = st_["hTbf"]
            nq = 48 if ti < 2 else 32
            c0 = 96 * ti
            fps = _t(ps_mm, [128, 512], F32, "mm")
            for ch in range(2):
                nc.tensor.matmul(out=fps[0:nq, 0:D],
                                 lhsT=sTbf_all[ch][:, c0 + dire:c0 + 2 * nq:2],
                                 rhs=Wf1_sb[ch][:],
                                 start=(ch == 0), stop=False)
            for ch in range(2):
                nc.tensor.matmul(out=fps[0:nq, 0:D],
                                 lhsT=hTbf[ch][:, 48 * ti:48 * ti + nq],
                                 rhs=Wf2_sb[ch][:],
                                 start=False, stop=(ch == 1))
            tsig = _t(work, [48, D], BF16, "tsig")
            nc.scalar.activation(tsig[0:nq, :], fps[0:nq, 0:D],
                                 AF.Tanh, scale=0.5)
            for ch in range(2):
                transpose_bf(fT_all[dire][ch][:, 48 * ti:48 * ti + nq],
                             tsig[0:nq, ch * DC:(ch + 1) * DC], nq, DC)

        def gate_final_pieces(blk, st_, sT_all, fT_all):
            """u from fT_all; att_s; cv — after all three tiles are in."""
            hT = st_["hT"]
            gs = {"uT": {}}

            def u_piece(dire):
                for ch in range(2):
                    fT = fT_all[dire][ch]
                    nc.vector.tensor_scalar(out=fT[:], in0=fT[:], scalar1=0.5,
                                            scalar2=0.5, op0=ALU.mult,
                                            op1=ALU.add)
                    sTv = sT_all[ch][:, dire::2]
                    dt_ = _t(work, [DC, L], F32, f"d{dire}{ch}")
                    nc.vector.tensor_sub(dt_[:], hT[ch][:], sTv)
                    nc.vector.tensor_mul(dt_[:], fT[:], dt_[:])
                    u = _t(blockp, [DC, L], F32, f"uT{blk}{dire}{ch}", bufs=1)
                    nc.vector.tensor_add(u[:], sTv, dt_[:])
                    ub = _t(blockp, [DC, L], BF16, f"uTb{blk}{dire}{ch}",
                            bufs=1)
                    nc.vector.tensor_copy(ub[:], u[:])
                    gs["uT"][(dire, ch)] = u
                    gs.setdefault("uTb", {})[(dire, ch)] = ub

            def ws1_piece():
                gs["uT_list"] = [gs["uT"][(0, 0)], gs["uT"][(0, 1)],
                                 gs["uT"][(1, 0)], gs["uT"][(1, 1)]]
                uTb_list = [gs["uTb"][(0, 0)], gs["uTb"][(0, 1)],
                            gs["uTb"][(1, 0)], gs["uTb"][(1, 1)]]
                wps = _t(ps_mm, [128, 512], F32, "mm")
                for q in range(4):
                    nc.tensor.matmul(out=wps[:, 0:2 * D], lhsT=uTb_list[q][:],
                                     rhs=Ws1_sb[q][:],
                                     start=(q == 0), stop=(q == 3))
                gs["w_sb"] = elu_from_psum_bf(wps[:, 0:2 * D], [L, 2 * D],
                                              "w")

            def wt_piece(q0):
                for q in (q0, q0 + 1):
                    dst = _t(work, [DC, L], BF16, f"wT{q}")
                    transpose_bf(dst[:], gs["w_sb"][:, q * DC:(q + 1) * DC],
                                 L, DC)
                    gs.setdefault("wT", {})[q] = dst

            def ws_piece():
                aps = _t(ps_mm, [128, 512], F32, "mm")
                for q in range(4):
                    nc.tensor.matmul(out=aps[:, 0:2 * D], lhsT=gs["wT"][q][:],
                                     rhs=Ws_sb[q][:],
                                     start=(q == 0), stop=(q == 3))
                atts_sb = _t(work, [L, 2 * D], F32, "atts")
                nc.scalar.copy(atts_sb[:], aps[:, 0:2 * D])
                gs["atts"] = atts_sb

            def cv_piece(q0):
                for q in (q0, q0 + 1):
                    aT = _t(ps_mm, [128, 512], F32, "mm")
                    nc.tensor.transpose(out=aT[0:DC, 0:L],
                                        in_=gs["atts"][:, q * DC:(q + 1) * DC],
                                        identity=identf_sb[:, :])
                    vT = _t(work, [DC, L], F32, "vT")
                    nc.vector.scalar_tensor_tensor(
                        vT[:], gs["uT_list"][q][:], 1.0, aT[0:DC, 0:L],
                        op0=ALU.mult, op1=ALU.mult,
                        accum_out=cv_sb[blk][:, q:q + 1])

            return [lambda: u_piece(0), lambda: u_piece(1),
                    ws1_piece, lambda: wt_piece(0), lambda: wt_piece(2),
                    ws_piece, lambda: cv_piece(0), lambda: cv_piece(2)]

        # ================== emission schedule ==================
        st_c = prep_block("c", bulk_hook=emit_bulk_loads)

        st_tiles = {}
        s_pk = {"c": {}, "r": {}}
        sT_all = {"c": [_t(blockp, [DC, 2 * L], F32, f"sTc{ch}", bufs=1)
                        for ch in range(2)],
                  "r": [_t(blockp, [DC, 2 * L], F32, f"sTr{ch}", bufs=1)
                        for ch in range(2)]}
        fT_all = {b: [[_t(blockp, [DC, L], BF16, f"fT{b}{dd}{ch}", bufs=1)
                       for ch in range(2)] for dd in range(2)]
                  for b in ("c", "r")}
        sTbf_all = {b: [_t(blockp, [DC, 2 * L], BF16, f"sTb{b}{ch}", bufs=1)
                        for ch in range(2)]
                    for b in ("c", "r")}

        # global chunk order: c0..c15, r0..r15; ST matmuls trail z-production
        # by SKEW chunks so the tensor queue always has ready work ahead of
        # the scalar/vector z chain.
        SKEW = 2
        seq = [("c", i) for i in range(N_CHUNKS)] + \
              [("r", i) for i in range(N_CHUNKS)]
        sts = {"c": st_c}

        def tiles_for(blk):
            if blk not in st_tiles:
                st_tiles[blk] = [_t(ps_st, [128, 512], F32, "st")
                                 for _ in range(3)]
            return st_tiles[blk]

        pending_gate = []
        for it in range(len(seq) + SKEW):
            if it == 2:
                # r prep interleaves with c's early main loop
                sts["r"] = prep_block("r")
            if it < len(seq):
                blk, ci = seq[it]
                emit_zprod(blk, sts[blk], ci)
            if it >= SKEW:
                blk, ci = seq[it - SKEW]
                st_ = sts[blk]
                emit_st(blk, st_, ci, tiles_for(blk))
                # tile boundary actions keyed on the just-emitted ST chunk
                if ci in (5, 11, 15):
                    ti = (5, 11, 15).index(ci)
                    emit_post(blk, st_, ti, st_tiles[blk], s_pk[blk])
                if ci in (6, 12):
                    ti = (6, 12).index(ci)
                    emit_s_transpose(blk, st_, ti, s_pk[blk], sT_all[blk],
                                     sTbf_all[blk])
                    pending_gate.append(
                        lambda b=blk, t=ti: gate_tile_piece(
                            b, sts[b], sT_all[b], sTbf_all[b],
                            fT_all[b], t, 0))
                    pending_gate.append(
                        lambda b=blk, t=ti: gate_tile_piece(
                            b, sts[b], sT_all[b], sTbf_all[b],
                            fT_all[b], t, 1))
                if ci == 15:
                    emit_s_transpose(blk, st_, 2, s_pk[blk], sT_all[blk],
                                     sTbf_all[blk])
                    pending_gate.append(
                        lambda b=blk: gate_tile_piece(
                            b, sts[b], sT_all[b], sTbf_all[b],
                            fT_all[b], 2, 0))
                    pending_gate.append(
                        lambda b=blk: gate_tile_piece(
                            b, sts[b], sT_all[b], sTbf_all[b],
                            fT_all[b], 2, 1))
                    pending_gate.extend(gate_final_pieces(
                        blk, sts[blk], sT_all[blk], fT_all[blk]))
                elif pending_gate:
                    pending_gate.pop(0)()
        while pending_gate:
            pending_gate.pop(0)()

        # ---------- head: feat = [cv, rv, cv-rv, cv*rv]; y ----------
        # feat columns as exact bf16 hi/lo weight pairs x bf16 F1, one
        # [2, 200] psum accumulation; then y = sum(relu(y1) * F2row).
        diff = _t(singles, [DC, 4], F32, "diff")
        nc.vector.tensor_sub(diff[:], cv_sb["c"][:], cv_sb["r"][:])
        prod = _t(singles, [DC, 4], F32, "prod")
        nc.vector.tensor_mul(prod[:], cv_sb["c"][:], cv_sb["r"][:])
        groups = [cv_sb["c"], cv_sb["r"], diff, prod]

        featp = _t(singles, [DC, 4, 4, 2], BF16, "featp")
        for gi, grp in enumerate(groups):
            nc.vector.tensor_copy(featp[:, gi, :, 0], grp[:])
            rem = _t(sml, [DC, 4], F32, "rem")
            nc.vector.tensor_sub(rem[:], grp[:], featp[:, gi, :, 0])
            nc.vector.tensor_copy(featp[:, gi, :, 1], rem[:])

        y1p = _t(ps_mm, [128, 512], F32, "mm")
        for kc in range(16):
            nc.tensor.matmul(out=y1p[0:2, 0:D],
                             lhsT=featp[:, kc // 4, kc % 4, :],
                             rhs=f1bf_sb[:, kc, :],
                             start=(kc == 0), stop=(kc == 15))
        y2sb = _t(sml, [2, D], F32, "y2sb")
        nc.scalar.copy(y2sb[:], y1p[0:2, 0:D])
        onesf2 = _t(sml, [2, 1], F32, "onesf2")
        nc.vector.memset(onesf2[:], 1.0)
        yrow = _t(ps_mm, [128, 512], F32, "mm")
        nc.tensor.matmul(out=yrow[0:1, 0:D], lhsT=onesf2[:], rhs=y2sb[:],
                         start=True, stop=True)
        r1 = _t(sml, [1, D], F32, "r1")
        nc.scalar.activation(r1[:], yrow[0:1, 0:D], AF.Relu)
        ym = _t(sml, [1, D], F32, "ym")
        nc.vector.tensor_mul(ym[:], r1[:], f2row_sb[:])
        y_sb = _t(sml, [1, 1], F32, "ysb")
        nc.vector.tensor_reduce(out=y_sb[:], in_=ym[:], axis=AX.X, op=ALU.add)
        nc.sync.dma_start(out=y_out, in_=y_sb[:])

    nc.compile()
    return nc


def _build_masks(ids):
    """[128, 256] bf16: col 2i+0 = fw col for query i (keys m>i), 2i+1 = bw
    (m<i); pad keys and pad queries zero the column."""
    np1 = (ids != PAD).astype(np.float32)
    m = np.arange(L)
    fw = (m[:, None] > m[None, :]).astype(np.float32) * np1[:, None] * np1[None, :]
    bw = (m[:, None] < m[None, :]).astype(np.float32) * np1[:, None] * np1[None, :]
    out = np.empty((L, 2 * L), np.float32)
    out[:, 0::2] = fw
    out[:, 1::2] = bw
    return out.astype(ml_dtypes.bfloat16)


def make_in_maps(inputs):
    x1 = np.asarray(inputs["x1"]).astype(np.int64)
    x2 = np.asarray(inputs["x2"]).astype(np.int64)
    f32 = lambda k: np.ascontiguousarray(np.asarray(inputs[k], np.float32))
    emb = f32("emb_w")
    shared = {
        "emb": emb,
        "Wh": f32("Wh_w"), "W1": f32("W1_w"), "W2": f32("W2_w"),
        "Wf1": f32("Wf1_w").astype(ml_dtypes.bfloat16),
        "Wf2": f32("Wf2_w").astype(ml_dtypes.bfloat16),
        "Ws1": f32("Ws1_w").astype(ml_dtypes.bfloat16),
        "Ws": f32("Ws_w").astype(ml_dtypes.bfloat16),
        "F1bf": np.ascontiguousarray(
            f32("F1_w").reshape(16, DC, D).transpose(1, 0, 2)
        ).astype(ml_dtypes.bfloat16),
        "F2row": f32("F2_w").reshape(1, D),
        "b_rep": np.tile(f32("b").reshape(1, D), (L, 1)),
        "ident_f": np.eye(L, dtype=np.float32),
        "ident_b": np.eye(L, dtype=np.float32).astype(ml_dtypes.bfloat16),
    }
    in_maps = []
    for bidx in range(N_CORES):
        m = dict(shared)
        m["xc_idx"] = x1[bidx].reshape(L, 1).astype(np.int32)
        m["xr_idx"] = x2[bidx].reshape(L, 1).astype(np.int32)
        m["masks_c"] = _build_masks(x1[bidx])
        m["masks_r"] = _build_masks(x2[bidx])
        in_maps.append(m)
    return in_maps


_NC_CACHE = {}


def get_nc():
    if "nc" not in _NC_CACHE:
        _NC_CACHE["nc"] = build_nc()
    return _NC_CACHE["nc"]


def kernel(**inputs) -> np.ndarray:
    from concourse.bass_utils import run_bass_kernel_spmd
    nc = get_nc()
    in_maps = make_in_maps(inputs)
    res = run_bass_kernel_spmd(nc, in_maps, list(range(N_CORES)))
    y = np.array([np.asarray(res.results[i]["y"]).reshape(-1)[0]
                  for i in range(N_CORES)], dtype=np.float32)
    return y
